# revision 15
# baseline (speedup 1.0000x reference)
"""Trainium2 Bass kernel for nn_BlurF: depthwise 4x4 blur (upfirdn2d pad=(2,1)).

Fast path (blur kernel [1,3,3,1]x[1,3,3,1]/16, which is what setup_inputs
produces): hybrid split by output row to balance engines against the
~312 GB/s per-core DMA roofline (33.6 MB fp16 in+out => ~108 us floor).

  - Rows [0,192): PE banded-matmul path. Separable conv as two PE passes
    with the data stationary; band matrices hold integer taps [1,3,3,1]
    (input is host-prescaled by 1/16, exact in fp16). PSUM banks pack 2
    channels per bank (pass2 q1 packs 4 via tile_position col-tiling) so
    the PSUM->SBUF fp16 copies amortize per-op overhead; copies split
    between DVE and ACT.
  - Rows [192,256): elementwise path. Channels in partitions, both spatial
    dims free: [1,3,3,1] = [1,1]*[1,1]*[1,1], so 3 vertical + 3 horizontal
    shifted tensor_adds on DVE (fp16 2x mode).

General kernels fall back to the SVD banded path (previous baseline).
"""

import numpy as np
import concourse.bacc as bacc
import concourse.mybir as mybir
from concourse.tile import TileContext
from concourse.bass_utils import run_bass_kernel_spmd

N_CORES = 8
C, H, W = 128, 256, 256
PRECISION = "fp16io"  # fallback path precision
R_PE = 192            # rows on the PE path; rows [R_PE, 256) on the EW path

_BUILD_CACHE = {}


# ---------------------------------------------------------------- fast path
K1 = np.array([1.0, 3.0, 3.0, 1.0])


def _is_blur(kern):
    k2d = np.outer(K1, K1) / 16.0
    return kern.shape == (4, 4) and np.allclose(kern, k2d, rtol=0, atol=1e-6)


def _band(taps, n):
    """B[s, s'] = taps[a] where s' = s + a - 1, a in 0..3, clipped to [0,n)."""
    B = np.zeros((n, n), dtype=np.float64)
    for a in range(4):
        lo = max(0, 1 - a)
        hi = min(n, n + 1 - a)
        s = np.arange(lo, hi)
        B[s, s + a - 1] = taps[a]
    return B


def _fast_bands():
    """Two band tiles in one [128, 512] f16 tensor.

    cols [0:256)   BD0 = B[0:128, 0:256]    (pass1 t=0 rhs; pass2 m=0 rhs)
    cols [256:512) BD1 = B[128:256, 0:256]  (pass2 m=1 rhs; [:,126:256] = pass1 t=1 rhs)
    """
    B = _band(K1, 256)
    bd = np.zeros((128, 512), dtype=np.float32)
    bd[:, 0:256] = B[0:128, 0:256]
    bd[:, 256:512] = B[128:256, 0:256]
    return bd.astype(np.float16)


FAST_CFG = dict(G=8, xin_bufs=2, vt_bufs=6, yout_bufs=2, p1_bufs=4,
                p2_bufs=4, out_engine="gpsimd", dma_split=2,
                p1_pack=2, p2_comb=False, defer=2, ew_rows=32, dve_share=3)


def _emit_fast(nc, tc, x, y, bd, cfg=None):
    cfg = {**FAST_CFG, **(cfg or {})}
    f16 = mybir.dt.float16
    f32 = mybir.dt.float32
    G = cfg["G"]
    NG = C // G
    PK = cfg["p1_pack"]            # channels per p1 tile (1 or 2 banks)
    P2C = cfg["p2_comb"]           # pass2: one 2-bank tile per pair (both q)
    E = cfg["ew_rows"]             # rows [R, 256) on the elementwise path
    R = 256 - E                    # rows [0, R) on the PE path
    K1 = min(R + 2, 256) - 128     # pass1 t=1 contraction depth
    N1 = R - 126                   # pass1 t=1 band width
    Q1 = R - 128                   # pass2 q=1 output rows
    p1_bufs = cfg["p1_bufs"] or (2 if PK == 4 else 3)
    p2_bufs = cfg["p2_bufs"] or (2 if P2C else (8 - p1_bufs * (PK // 2)))
    out_dma = {"gpsimd": nc.gpsimd, "scalar": nc.scalar,
               "sync": nc.sync}[cfg["out_engine"]]
    with (
        tc.tile_pool(name="bands", bufs=1) as band_pool,
        tc.tile_pool(name="xin0", bufs=cfg["xin_bufs"]) as xin0_pool,
        tc.tile_pool(name="xin1", bufs=cfg["xin_bufs"]) as xin1_pool,
        tc.tile_pool(name="vts", bufs=cfg["vt_bufs"]) as vt_pool,
        tc.tile_pool(name="yout", bufs=cfg["yout_bufs"]) as yout_pool,
        tc.tile_pool(name="p1", bufs=p1_bufs, space="PSUM") as p1_pool,
        tc.tile_pool(name="p2", bufs=p2_bufs, space="PSUM") as p2_pool,
        tc.tile_pool(name="ewin", bufs=2) as ewin_pool,
        tc.tile_pool(name="ewv", bufs=1) as ewv_pool,
        tc.tile_pool(name="ewout", bufs=2) as ewout_pool,
    ):
        bds = band_pool.tile([128, 512], f16, tag="bd")
        nc.sync.dma_start(out=bds[:], in_=bd[:, :])
        BD0 = bds[:, 0:256]
        BD1 = bds[:, 256:512]
        BD0v = bds[:, 0:R]
        BD1c = bds[0:K1, 256 + 126:256 + R]
        nco = [0]  # running copy-op counter for weighted engine balance

        def copy_op(dst, src):
            if nco[0] % 7 < cfg["dve_share"]:
                nc.vector.tensor_copy(dst, src)
            else:
                nc.scalar.copy(dst, src)
            nco[0] += 1

        ew_state = {}

        def ew_step(i):
            """One step of the EW chain per call; DVE ops stay interleaved
            with copies in the DVE FIFO."""
            s = ew_state
            if i == 0:
                f = ewin_pool.tile([128, E + 3, 260], f16, tag="ewin")
                nc.gpsimd.memset(f[:, :, 0:2], 0.0)
                nc.gpsimd.memset(f[:, :, 258:260], 0.0)
                nc.gpsimd.memset(f[:, E + 2:E + 3, :], 0.0)
                nc.sync.dma_start(out=f[:, 0:E + 2, 2:258],
                                  in_=x[R - 2:256, :, :]
                                  .rearrange("y c x -> c y x"))
                s["f"] = f
            elif i == 1:
                v1 = ewv_pool.tile([128, E + 2, 260], f16, tag="v1")
                f = s["f"]
                nc.vector.tensor_add(v1[:], f[:, 0:E + 2, :], f[:, 1:E + 3, :])
                s["v1"] = v1
            elif i == 2:
                v2 = ewv_pool.tile([128, E + 1, 260], f16, tag="v2")
                v1 = s["v1"]
                nc.vector.tensor_add(v2[:], v1[:, 0:E + 1, :], v1[:, 1:E + 2, :])
                s["v2"] = v2
            elif i == 3:
                v3 = ewv_pool.tile([128, E, 260], f16, tag="v3")
                v2 = s["v2"]
                nc.vector.tensor_add(v3[:], v2[:, 0:E, :], v2[:, 1:E + 1, :])
                s["v3"] = v3
            elif i == 4:
                h1 = ewv_pool.tile([128, E, 258], f16, tag="h1")
                v3 = s["v3"]
                nc.vector.tensor_add(h1[:], v3[:, :, 0:258], v3[:, :, 1:259])
                s["h1"] = h1
            elif i == 5:
                h2 = ewv_pool.tile([128, E, 257], f16, tag="h2")
                h1 = s["h1"]
                nc.vector.tensor_add(h2[:], h1[:, :, 0:257], h1[:, :, 1:258])
                s["h2"] = h2
            elif i == 6:
                oew = ewout_pool.tile([128, E, 256], f16, tag="oew")
                h2 = s["h2"]
                nc.vector.tensor_add(oew[:], h2[:, :, 0:256], h2[:, :, 1:257])
                nc.gpsimd.dma_start(out=y[R:256, :, :]
                                    .rearrange("y c x -> c y x"), in_=oew[:])

        pend = []

        def emit_outdma(yout_t, c0_):
            for q in (0, 1):
                rows = 128 if q == 0 else Q1
                for h in range(cfg["dma_split"]):
                    gsz = G // cfg["dma_split"]
                    cs = c0_ + h * gsz
                    out_dma.dma_start(
                        out=y[q * 128:q * 128 + rows, cs:cs + gsz, :],
                        in_=yout_t[0:rows, h * gsz:(h + 1) * gsz, q, :])

        def emit_pass2(vts, yout_t, b, dma_c0):
            for pp in range(PK // 2):
                j0 = b * PK + pp * 2
                vsl = [vts[m][:, pp * 2 * R:(pp + 1) * 2 * R] for m in (0, 1)]
                if P2C:
                    p2 = p2_pool.tile([128, 1024], f32, tag="p2")
                    for c2 in (0, 1):
                        for q in (0, 1):
                            for m in (0, 1):
                                nc.tensor.matmul(
                                    p2[:, c2 * 512 + q * 256:
                                       c2 * 512 + q * 256 + 256],
                                    vsl[m][:, c2 * 256 + q * 128:
                                           c2 * 256 + q * 128 + 128],
                                    BD0 if m == 0 else BD1,
                                    start=(m == 0), stop=(m == 1))
                    copy_op(yout_t[:, j0:j0 + 2, :, :], p2[:])
                else:
                    for q in (0, 1):
                        rows = 128 if q == 0 else Q1
                        p2 = p2_pool.tile([128, 512], f32, tag="p2")
                        for c2 in (0, 1):
                            for m in (0, 1):
                                nc.tensor.matmul(
                                    p2[0:rows, c2 * 256:(c2 + 1) * 256],
                                    vsl[m][:, c2 * R + q * 128:
                                           c2 * R + q * 128 + rows],
                                    BD0 if m == 0 else BD1,
                                    start=(m == 0), stop=(m == 1))
                        copy_op(yout_t[0:rows, j0:j0 + 2, q, :],
                                p2[0:rows, :])
            if dma_c0 is not None:
                emit_outdma(yout_t, dma_c0)

        for g in range(NG):
            c0 = g * G
            xins = []
            for t in (0, 1):
                kd = 128 if t == 0 else K1
                xt = (xin0_pool if t == 0 else xin1_pool).tile(
                    [kd, G, 256], f16, tag=f"xin{t}", name=f"xin{t}")
                for h in range(cfg["dma_split"]):
                    gsz = G // cfg["dma_split"]
                    cs = c0 + h * gsz
                    nc.sync.dma_start(
                        out=xt[:, h * gsz:(h + 1) * gsz, :],
                        in_=x[t * 128:t * 128 + kd, cs:cs + gsz, :])
                xins.append(xt)
            yout = yout_pool.tile([128, G, 2, 256], f16, tag="youtc",
                                  name="youtc")
            # pass1 over PK-channel blocks; pass2 deferred by `defer` blocks
            # so the PE FIFO always has independent pass1 work while copies
            # of the previous block are in flight.
            for b in range(G // PK):
                vts = []
                for m in (0, 1):
                    p1 = p1_pool.tile([128, R * PK], f32, tag="p1")
                    for cc in range(PK):
                        j = b * PK + cc
                        nc.tensor.matmul(
                            p1[:, cc * R:(cc + 1) * R],
                            xins[0][:, j, m * 128:(m + 1) * 128],
                            BD0v, start=True, stop=False)
                        nc.tensor.matmul(
                            p1[:, cc * R + 126:(cc + 1) * R],
                            xins[1][0:K1, j, m * 128:(m + 1) * 128],
                            BD1c, start=False, stop=True)
                    v = vt_pool.tile([128, R * PK], f16, tag=f"vt{m}",
                                     name=f"vt{m}")
                    copy_op(v[:], p1[:])
                    vts.append(v)
                pend.append((vts, yout, b, c0 if b == G // PK - 1 else None))
                if len(pend) > cfg["defer"]:
                    emit_pass2(*pend.pop(0))
            if E and 4 <= g < 11:
                ew_step(g - 4)
            if g == NG - 1:
                while pend:
                    emit_pass2(*pend.pop(0))


def _build_fast(reps=1, loop_reps=None, cfg=None):
    key = ("fast", reps, loop_reps, tuple(sorted((cfg or {}).items())))
    if key in _BUILD_CACHE:
        return _BUILD_CACHE[key]
    f16 = mybir.dt.float16
    nc = bacc.Bacc("TRN2", target_bir_lowering=False, debug=False)
    x = nc.dram_tensor("x", [H, C, W], f16, kind="ExternalInput").ap()
    bd = nc.dram_tensor("bd", [128, 512], f16, kind="ExternalInput").ap()
    y = nc.dram_tensor("y", [H, C, W], f16, kind="ExternalOutput").ap()
    with TileContext(nc) as tc:
        if loop_reps is not None:
            with tc.For_i(0, loop_reps, 1):
                _emit_fast(nc, tc, x, y, bd, cfg)
        else:
            for _ in range(reps):
                _emit_fast(nc, tc, x, y, bd, cfg)
    nc.compile()
    _BUILD_CACHE[key] = nc
    return nc


def _prep_fast(fmap):
    bd = _fast_bands()
    in_maps = []
    for i in range(N_CORES):
        x16 = (np.asarray(fmap[i], dtype=np.float32) * (1.0 / 16.0)
               ).astype(np.float16)
        in_maps.append({"x": np.ascontiguousarray(x16.transpose(1, 0, 2)),
                        "bd": bd})
    return in_maps


# ------------------------------------------------------- fallback (SVD) path
def _round_f32r(a):
    b = np.ascontiguousarray(a, dtype=np.float32).view(np.uint32)
    b = (b + np.uint32(0x800)) & np.uint32(0xFFFFF000)
    return b.view(np.float32)


def _factorize(kernel4x4):
    k = np.asarray(kernel4x4, dtype=np.float64)
    U, S, Vt = np.linalg.svd(k)
    comps = []
    for r in range(4):
        if S[r] > 1e-9 * max(S[0], 1e-30):
            comps.append((U[:, r] * np.sqrt(S[r]), Vt[r, :] * np.sqrt(S[r])))
    return comps


DEFAULT_CFG = dict(
    G=16, out_engine="scalar", dma_split=2,
    xin_bufs=2, vt_bufs=3, yout_bufs=2, p1_bufs=3, p2_bufs=3,
)


def _emit(nc, tc, x, y, bvt, bht, rank, precision, cfg=None):
    cfg = {**DEFAULT_CFG, **(cfg or {})}
    Gc = cfg["G"]
    f32 = mybir.dt.float32
    f32r = mybir.dt.float32r
    mmdt = {"fp32": f32, "fp16": mybir.dt.float16,
            "fp16io": mybir.dt.float16}.get(precision, f32r)
    ydt = mybir.dt.float16 if precision == "fp16io" else f32
    split = precision == "fp32r_split"
    parts = (0, 1) if split else (0,)
    NG = C // Gc
    out_dma = nc.scalar if cfg["out_engine"] == "scalar" else nc.sync
    with (
        tc.tile_pool(name="xin", bufs=cfg["xin_bufs"]) as xin_pool,
        tc.tile_pool(name="vt", bufs=cfg["vt_bufs"]) as vt_pool,
        tc.tile_pool(name="yout", bufs=cfg["yout_bufs"]) as yout_pool,
        tc.tile_pool(name="p1", bufs=cfg["p1_bufs"], space="PSUM") as p1_pool,
        tc.tile_pool(name="p2", bufs=cfg["p2_bufs"], space="PSUM") as p2_pool,
    ):
        pending = [None]

        def emit_pass2(p):
            vts, youts, j, g = p
            ops = [(r, m, s) for r in range(rank) for m in (0, 1) for s in parts]
            for q in (0, 1):
                p2 = p2_pool.tile([128, 256], f32, tag="p2")
                for i, (r, m, s) in enumerate(ops):
                    nc.tensor.matmul(
                        p2[:],
                        vts[(r, m, s)][:, q * 128:(q + 1) * 128],
                        bht[r][m][:],
                        start=(i == 0),
                        stop=(i == len(ops) - 1),
                    )
                if q == 0:
                    nc.vector.tensor_copy(youts[q][:, j, :], p2[:])
                else:
                    nc.scalar.copy(youts[q][:, j, :], p2[:])
            ds = cfg["dma_split"]
            gsz = Gc // ds
            if (j + 1) % gsz == 0:
                h = (j + 1) // gsz - 1
                c0 = g * Gc + h * gsz
                for q in (0, 1):
                    out_dma.dma_start(
                        out=y[c0:c0 + gsz, q * 128:(q + 1) * 128, :]
                        .rearrange("c y x -> y c x"),
                        in_=youts[q][:, h * gsz:(h + 1) * gsz, :],
                    )

        for g in range(NG):
            xraw = []
            ds = cfg["dma_split"]
            gsz = Gc // ds
            for t in (0, 1):
                xt = xin_pool.tile([128, Gc, 256], f32 if split else mmdt,
                                   tag=f"xin{t}", name=f"xin{t}")
                for h in range(ds):
                    c0 = g * Gc + h * gsz
                    nc.sync.dma_start(
                        out=xt[:, h * gsz:(h + 1) * gsz, :],
                        in_=x[c0:c0 + gsz, t * 128:(t + 1) * 128, :]
                        .rearrange("c y x -> y c x"),
                    )
                xraw.append(xt)
            if split:
                xins = {}
                for t in (0, 1):
                    hi = xin_pool.tile([128, Gc, 256], f32r, tag=f"xhi{t}", name=f"xhi{t}")
                    nc.scalar.copy(hi[:], xraw[t][:])
                    lo = xin_pool.tile([128, Gc, 256], f32r, tag=f"xlo{t}", name=f"xlo{t}")
                    nc.vector.tensor_sub(lo[:], xraw[t][:], hi[:])
                    xins[(t, 0)] = hi
                    xins[(t, 1)] = lo
            else:
                xins = {(t, 0): xraw[t] for t in (0, 1)}
            youts = {
                q: yout_pool.tile([128, Gc, 256], ydt, tag=f"yout{q}", name=f"yout{q}")
                for q in (0, 1)
            }
            for j in range(Gc):
                vts = {}
                for m in (0, 1):
                    for r in range(rank):
                        p1 = p1_pool.tile([128, 256], f32, tag="p1")
                        mmops = [(t, s) for t in (0, 1) for s in parts]
                        for i, (t, s) in enumerate(mmops):
                            nc.tensor.matmul(
                                p1[:],
                                xins[(t, s)][:, j, m * 128:(m + 1) * 128],
                                bvt[r][t][:],
                                start=(i == 0),
                                stop=(i == len(mmops) - 1),
                            )
                        if split:
                            vhi = vt_pool.tile([128, 256], f32r,
                                               tag=f"vth{m}_{r}", name=f"vth{m}_{r}")
                            nc.scalar.copy(vhi[:], p1[:])
                            vlo = vt_pool.tile([128, 256], f32r,
                                               tag=f"vtl{m}_{r}", name=f"vtl{m}_{r}")
                            nc.vector.tensor_sub(vlo[:], p1[:], vhi[:])
                            vts[(r, m, 0)] = vhi
                            vts[(r, m, 1)] = vlo
                        else:
                            v = vt_pool.tile([128, 256], mmdt,
                                             tag=f"vt{m}_{r}", name=f"vt{m}_{r}")
                            if m == 0:
                                nc.vector.tensor_copy(v[:], p1[:])
                            else:
                                nc.scalar.copy(v[:], p1[:])
                            vts[(r, m, 0)] = v
                if pending[0] is not None:
                    emit_pass2(pending[0])
                pending[0] = (vts, youts, j, g)
        emit_pass2(pending[0])


def _build(rank, precision, reps=1, loop_reps=None, cfg=None):
    key = (rank, precision, reps, loop_reps,
           tuple(sorted((cfg or {}).items())))
    if key in _BUILD_CACHE:
        return _BUILD_CACHE[key]
    f32 = mybir.dt.float32
    mmdt = {"fp32": f32, "fp16": mybir.dt.float16,
            "fp16io": mybir.dt.float16}.get(precision, mybir.dt.float32r)
    xdt = f32 if precision in ("fp32", "fp32r_split") else mmdt
    ydt = mybir.dt.float16 if precision == "fp16io" else f32
    nc = bacc.Bacc("TRN2", target_bir_lowering=False, debug=False)
    x = nc.dram_tensor("x", [C, H, W], xdt, kind="ExternalInput").ap()
    bv = nc.dram_tensor("bv", [rank, 2, 128, 256], mmdt, kind="ExternalInput").ap()
    bh = nc.dram_tensor("bh", [rank, 2, 128, 256], mmdt, kind="ExternalInput").ap()
    y = nc.dram_tensor("y", [C, H, W], ydt, kind="ExternalOutput").ap()
    with TileContext(nc) as tc:
        with tc.tile_pool(name="bands", bufs=1) as band_pool:
            bvt = [[None, None] for _ in range(rank)]
            bht = [[None, None] for _ in range(rank)]
            for r in range(rank):
                for t in (0, 1):
                    bvt[r][t] = band_pool.tile([128, 256], mmdt, tag=f"bv{r}{t}", name=f"bv{r}{t}")
                    nc.sync.dma_start(out=bvt[r][t][:], in_=bv[r, t])
                    bht[r][t] = band_pool.tile([128, 256], mmdt, tag=f"bh{r}{t}", name=f"bh{r}{t}")
                    nc.sync.dma_start(out=bht[r][t][:], in_=bh[r, t])
            if loop_reps is not None:
                with tc.For_i(0, loop_reps, 1):
                    _emit(nc, tc, x, y, bvt, bht, rank, precision, cfg)
            else:
                for _ in range(reps):
                    _emit(nc, tc, x, y, bvt, bht, rank, precision, cfg)
    nc.compile()
    _BUILD_CACHE[key] = nc
    return nc


def _band_f(taps, n):
    return _band(taps, n)


def _prep_inputs(fmap, kernel4x4, precision):
    comps = _factorize(kernel4x4)
    rank = max(1, len(comps))
    while len(comps) < rank:
        comps.append((np.zeros(4), np.zeros(4)))
    bv = np.zeros((rank, 2, 128, 256), dtype=np.float32)
    bh = np.zeros((rank, 2, 128, 256), dtype=np.float32)
    for r, (u, v) in enumerate(comps):
        Bv = _band(u, H).astype(np.float32)
        Bh = _band(v, W).astype(np.float32)
        bv[r] = Bv.reshape(2, 128, 256)
        bh[r] = Bh.reshape(2, 128, 256)
    if precision in ("fp32r", "fp32r_split"):
        bv, bh = _round_f32r(bv), _round_f32r(bh)
    elif precision in ("fp16", "fp16io"):
        bv, bh = bv.astype(np.float16), bh.astype(np.float16)
    in_maps = []
    for i in range(N_CORES):
        shard = np.ascontiguousarray(fmap[i], dtype=np.float32)
        if precision == "fp32r":
            shard = _round_f32r(shard)
        elif precision in ("fp16", "fp16io"):
            shard = shard.astype(np.float16)
        in_maps.append({"x": shard, "bv": bv, "bh": bh})
    return rank, in_maps


def _run(nc, in_maps):
    last_err = None
    for _attempt in range(3):
        try:
            return run_bass_kernel_spmd(nc, in_maps, list(range(N_CORES)),
                                        trace=False)
        except Exception as e:
            last_err = e
            import time
            time.sleep(2.0)
    raise last_err


def kernel(fmap, kernel):
    fmap = np.asarray(fmap)
    kern = np.asarray(kernel)
    assert fmap.shape == (N_CORES, C, H, W), fmap.shape
    if _is_blur(kern):
        in_maps = _prep_fast(fmap)
        nc = _build_fast()
        res = _run(nc, in_maps)
    else:
        rank, in_maps = _prep_inputs(fmap, kern, PRECISION)
        nc = _build(rank, PRECISION)
        res = _run(nc, in_maps)
    outs = [res.results[i]["y"] for i in range(N_CORES)]
    if _is_blur(kern):
        outs = [o.transpose(1, 0, 2) for o in outs]
    out = np.stack(outs, axis=0)
    return np.ascontiguousarray(out.astype(np.float32))
